# revision 29
# baseline (speedup 1.0000x reference)
"""Trainium2 Bass kernel for a Matching Network attention head (v6).

Reference computation:
    q_proj = query @ W1[:D]                       # [Q, D]
    s_proj = support @ W1[D:]                     # [S, D]
    hidden = relu(q_proj[:,None,:] + s_proj[None,:,:] + b1)   # [Q, S, D]
    scores = einsum('qsd,d->qs', hidden, W2) + b2
    weights = softmax(scores, axis=1)
    logits  = weights @ onehot(support_labels)    # [Q, n_way]

Sharding (8 cores): shard the SUPPORT set (40 rows/core), replicate
queries.  Each core emits unnormalized softmax partials
    part[w, q]  = sum_{s in shard} exp(score[s,q]) * onehot[s,w]
    part[20, q] = sum_{s in shard} exp(score[s,q])
summed and divided on the host (b2 cancels in the softmax; exp without
max-subtraction is safe: scores ~ N(0,1)).

Measured model driving the v6 schedule (see trace notes in repo memory):
  - main loop is engine-saturated at ~40us: 60 DVE relu tiles
    (tensor_scalar ADD,MAX bf16 4x: ~615ns effective) + 20 ACT relu
    tiles (~1950ns) — the wins are startup and tail latency.
  - DMA: each entry sustains only ~60-95GB/s; sync ring aggregates
    ~340GB/s over concurrent entries, scalar ring caps ~90GB/s total.
    So the startup-critical bytes ride sync as MANY SMALL entries in
    dependency-priority order and the leftovers ride scalar.
  - PE HAM: ~3.4us of sustained activity un-throttles 1.2->2.4GHz;
    idle windows re-throttle.  Warmup dummies bridge the DMA wait, and
    tail warm-keepers (dependency-pinned to late H tiles) keep the
    last-round matmuls warm.
  - startup: qpT psum->sbuf copies interleave with round-0 chunked H
    ops per q-chunk so the DVE queue never head-of-line blocks.
  - tail: only the last round's db0 half remains at the end (db1 was
    pulled into round 8), produced in two half-width chunks so the
    first exp fires ~2us before the last H op; per-qc
    exp -> final matmul -> copy -> DMA, copies on DVE (idle), last on
    ACT, DMAs on the idle sync ring.
"""

import numpy as np
import ml_dtypes

bf16 = ml_dtypes.bfloat16

N_CORES = 8
Q, D, S, NWAY = 2048, 256, 320, 20
SP = S // N_CORES          # 40 support rows per core
NQC = 4                    # q chunks of 512 (one psum bank each)
QC = Q // NQC
NR = SP // 4               # 10 rounds of 4 concurrent s-values

# const-blob column layout (bf16, [128, NB])
OFF_W1B = 0                # [128, 256] x2 (din block major)
OFF_ST = 512               # [128, 40] x2
OFF_B1F = 592              # [128, 2]: col db = b1[128*db:128*(db+1)]
OFF_W1A = 594              # [128, 256] x2
END_PA = 1106              # end of critical blob piece
OFF_W2C = 1106             # [128, 640]: 2 dblk x 10 rounds x [128, 32]
OFF_OHM = 1746             # [128, 21] x2 (ohmA | ohmB)
NB = 1788

N_WARM_PRE = 10            # PE warmup matmuls bridging the DMA wait

_compiled = None


def _act_tile_set():
    """(j, db) -> round assignment of H tiles to the ACT engine.

    20 of the 72 round-0..8 tiles go to ACT (the last round's
    remaining db0 half is always DVE)."""
    act = set()
    ts_idx = 0
    for r in range(NR - 1):
        for j in range(4):
            for db in range(2):
                if (ts_idx * 20) % 72 < 20:
                    act.add((r, j, db))
                ts_idx += 1
    return act


def _build_nc():
    import concourse.tile as tile
    from concourse import mybir
    from concourse.bacc import Bacc

    f32 = mybir.dt.float32
    b16 = mybir.dt.bfloat16
    RELU = mybir.ActivationFunctionType.Relu
    EXP = mybir.ActivationFunctionType.Exp
    IDENT = mybir.ActivationFunctionType.Identity
    ADD = mybir.AluOpType.add
    MAX = mybir.AluOpType.max

    nc = Bacc()
    blob_d = nc.declare_dram_parameter("blob", [128, NB], b16, isOutput=False)
    qT_d = nc.declare_dram_parameter("qT", [D, Q], b16, isOutput=False)
    out_d = nc.declare_dram_parameter("part", [NWAY + 1, Q], b16, isOutput=True)

    ACT_SET = _act_tile_set()

    with tile.TileContext(nc) as tc:
        with (
            tc.tile_pool(name="const", bufs=1) as cpool,
            tc.tile_pool(name="stage", bufs=1) as spool,
            tc.tile_pool(name="hpool", bufs=16) as hpool,
            tc.tile_pool(name="psum", bufs=8, space="PSUM") as ppool,
        ):
            # ---- inputs ----------------------------------------------
            blob_t = cpool.tile([128, NB], b16, name="blobt")
            qT_t = [spool.tile([128, Q], b16, name=f"qTt{i}") for i in range(2)]
            scratch_t = cpool.tile([128, 256], b16, name="scratch")
            b1f_t = cpool.tile([128, 2], f32, name="b1f")

            # sync ring: critical pieces, small entries, priority order
            nc.sync.dma_start(out=blob_t[:, 0:OFF_W1A], in_=blob_d[:, 0:OFF_W1A])
            nc.sync.dma_start(
                out=blob_t[:, OFF_W1A:END_PA], in_=blob_d[:, OFF_W1A:END_PA]
            )
            for qc in range(3):
                for i in range(2):
                    nc.sync.dma_start(
                        out=qT_t[i][:, QC * qc : QC * (qc + 1)],
                        in_=qT_d[128 * i : 128 * (i + 1), QC * qc : QC * (qc + 1)],
                    )
            # scalar ring (slow): W2C/ohm + the last q-chunk
            nc.scalar.dma_start(out=blob_t[:, END_PA:NB], in_=blob_d[:, END_PA:NB])
            for i in range(2):
                nc.scalar.dma_start(
                    out=qT_t[i][:, QC * 3 : Q],
                    in_=qT_d[128 * i : 128 * (i + 1), QC * 3 : Q],
                )

            def w1a(dinb, doutb):
                o = OFF_W1A + 256 * dinb + 128 * doutb
                return blob_t[:, o : o + 128]

            def w1b(dinb, doutb):
                o = OFF_W1B + 256 * dinb + 128 * doutb
                return blob_t[:, o : o + 128]

            def sT(dinb):
                o = OFF_ST + SP * dinb
                return blob_t[:, o : o + SP]

            def w2col(db, r):
                o = OFF_W2C + 32 * (db * NR + r)
                return blob_t[:, o : o + 32]

            def ohm(half):
                o = OFF_OHM + (NWAY + 1) * half
                return blob_t[:, o : o + NWAY + 1]

            # ---- PE warmup (no DMA deps): un-throttle HAM early ------
            nc.vector.memset(scratch_t[:], 0.0)
            warm_ps = [
                ppool.tile([128, 256], f32, tag="ps", name=f"warmps{i}")
                for i in range(2)
            ]
            for i in range(N_WARM_PRE):
                nc.tensor.matmul(
                    warm_ps[i % 2][:], scratch_t[:, 0:128], scratch_t[:],
                    start=True, stop=True,
                )

            # b1 (bf16 cols in blob) -> fp32 for activation bias
            nc.vector.tensor_copy(out=b1f_t[:], in_=blob_t[:, OFF_B1F : OFF_B1F + 2])

            # ---- qpT production interleaved with round-0 H chunks ----
            # qc0's qpT is emitted BEFORE spb (its DMA lands first; spb
            # waiting on the later blob piece must not head-of-line
            # block it on the PE queue).  DVE queue: cast(qc0) -> r0
            # qc0 chunks -> r0 qc1-3.  All other qpT copies ride ACT.
            qpT_t = [spool.tile([128, Q], b16, name=f"qpT{i}") for i in range(2)]
            spb_t = [cpool.tile([128, SP], f32, name=f"spb{i}") for i in range(2)]
            r0_dve = [(j, db) for j in range(4) for db in range(2)
                      if (0, j, db) not in ACT_SET]
            h0_tiles = {}
            for (j, db) in r0_dve:
                h0_tiles[(j, db)] = hpool.tile(
                    [128, Q], b16, tag="Hd", bufs=26, name=f"h{j}_{db}"
                )

            def emit_r0_chunk(lo, hi):
                for (j, db) in r0_dve:
                    nc.vector.tensor_scalar(
                        out=h0_tiles[(j, db)][:, lo:hi],
                        in0=qpT_t[db][:, lo:hi],
                        scalar1=spb_t[db][:, j : j + 1],
                        scalar2=0.0, op0=ADD, op1=MAX,
                    )

            def emit_qpT(qc):
                for db in range(2):
                    qps = ppool.tile([128, QC], f32, tag="ps", name=f"qps{db}{qc}")
                    nc.tensor.matmul(
                        qps[:], w1a(0, db), qT_t[0][:, QC * qc : QC * (qc + 1)],
                        start=True, stop=False,
                    )
                    nc.tensor.matmul(
                        qps[:], w1a(1, db), qT_t[1][:, QC * qc : QC * (qc + 1)],
                        start=False, stop=True,
                    )
                    dst = qpT_t[db][:, QC * qc : QC * (qc + 1)]
                    if qc == 0 and db == 0:
                        nc.vector.tensor_copy(out=dst, in_=qps[:])
                    else:
                        nc.scalar.copy(out=dst, in_=qps[:])

            emit_qpT(0)

            # ---- spbT = W1b^T @ supportT (+b1 on the copy)  [2][128,SP]
            for db in range(2):
                sps = ppool.tile([128, QC], f32, tag="ps", name=f"sps{db}")
                nc.tensor.matmul(sps[:, :SP], w1b(0, db), sT(0), start=True, stop=False)
                nc.tensor.matmul(sps[:, :SP], w1b(1, db), sT(1), start=False, stop=True)
                nc.scalar.activation(
                    spb_t[db][:], sps[:, :SP], IDENT, bias=b1f_t[:, db : db + 1]
                )

            emit_r0_chunk(0, QC)
            for qc in range(1, NQC):
                emit_qpT(qc)
            emit_r0_chunk(QC, 2 * QC)
            emit_r0_chunk(2 * QC, Q)

            # ---- main loop -------------------------------------------
            e_t = spool.tile([128, Q], b16, name="et")
            out_sb = spool.tile([NWAY + 1, Q], b16, name="outsb")
            scores_ps = [
                ppool.tile([128, QC], f32, tag="ps", name=f"sc{qc}")
                for qc in range(NQC)
            ]

            for r in range(NR - 1):
                h_tiles = {}
                for j in range(4):
                    sl = 4 * r + j
                    for db in range(2):
                        if r == 0:
                            h = h0_tiles[(j, db)] if (j, db) in r0_dve else None
                        else:
                            h = None
                        if h is None:
                            if (r, j, db) in ACT_SET:
                                h = hpool.tile(
                                    [128, Q], b16, tag="Ha", bufs=8, name=f"h{sl}_{db}"
                                )
                                nc.scalar.activation(
                                    h[:], qpT_t[db][:], RELU,
                                    bias=spb_t[db][:, sl : sl + 1],
                                )
                            else:
                                h = hpool.tile(
                                    [128, Q], b16, tag="Hd", bufs=26, name=f"h{sl}_{db}"
                                )
                                nc.vector.tensor_scalar(
                                    out=h[:], in0=qpT_t[db][:],
                                    scalar1=spb_t[db][:, sl : sl + 1],
                                    scalar2=0.0, op0=ADD, op1=MAX,
                                )
                        h_tiles[(j, db)] = h
                for db in range(2):
                    for qc in range(NQC):
                        for j in range(4):
                            nc.tensor.matmul(
                                scores_ps[qc][32 * j : 32 * j + 32, :],
                                w2col(db, r),
                                h_tiles[(j, db)][:, QC * qc : QC * (qc + 1)],
                                start=(r == 0 and db == 0),
                                stop=False,
                                tile_position=(0, 32 * j),
                                skip_group_check=True,
                            )
                if r == NR - 2:
                    # pull the LAST round's db1 half forward so only 4
                    # H tiles + 4 matmul groups gate the tail.
                    h9 = []
                    for j in range(4):
                        sl = 4 * (NR - 1) + j
                        h = hpool.tile(
                            [128, Q], b16, tag="Hd", bufs=26, name=f"h{sl}_1"
                        )
                        nc.vector.tensor_scalar(
                            out=h[:], in0=qpT_t[1][:],
                            scalar1=spb_t[1][:, sl : sl + 1],
                            scalar2=0.0, op0=ADD, op1=MAX,
                        )
                        h9.append(h)
                    for qc in range(NQC):
                        for j in range(4):
                            nc.tensor.matmul(
                                scores_ps[qc][32 * j : 32 * j + 32, :],
                                w2col(1, NR - 1),
                                h9[j][:, QC * qc : QC * (qc + 1)],
                                start=False,
                                stop=False,
                                tile_position=(0, 32 * j),
                                skip_group_check=True,
                            )

            # ---- last round (db0 only) + per-qc tail -----------------
            # H in three chunks [qc01 | qc2 | qc3] so qc0-qc2's scores
            # close (and their exps start) before the last H columns.
            r = NR - 1
            hl_tiles = {}
            for j in range(4):
                hl_tiles[j] = hpool.tile(
                    [128, Q], b16, tag="Hd", bufs=26, name=f"h{4 * r + j}_0"
                )
            for (lo, hi) in ((0, 2 * QC), (2 * QC, 3 * QC), (3 * QC, Q)):
                for j in range(4):
                    sl = 4 * r + j
                    nc.vector.tensor_scalar(
                        out=hl_tiles[j][:, lo:hi], in0=qpT_t[0][:, lo:hi],
                        scalar1=spb_t[0][:, sl : sl + 1],
                        scalar2=0.0, op0=ADD, op1=MAX,
                    )
                if lo == 0:
                    # PE warm-keepers pinned to the first chunks (can't
                    # be hoisted): keep HAM at 8/8 into the tail.
                    for i in range(2):
                        wp = ppool.tile([128, 256], f32, tag="ps", name=f"tw{i}")
                        nc.tensor.matmul(
                            wp[:],
                            hl_tiles[2 * i][:, 0:128],
                            hl_tiles[2 * i][:, 0:256],
                            start=True, stop=True,
                        )
            for qc in range(NQC):
                for j in range(4):
                    nc.tensor.matmul(
                        scores_ps[qc][32 * j : 32 * j + 32, :],
                        w2col(0, r),
                        hl_tiles[j][:, QC * qc : QC * (qc + 1)],
                        start=False,
                        stop=(j == 3),
                        tile_position=(0, 32 * j),
                        skip_group_check=True,
                    )
                nc.scalar.activation(
                    e_t[:, QC * qc : QC * (qc + 1)], scores_ps[qc][:], EXP,
                )
                fps = ppool.tile([NWAY + 1, QC], f32, tag="ps", name=f"fps{qc}")
                nc.tensor.matmul(
                    fps[:], ohm(0), e_t[:, QC * qc : QC * (qc + 1)],
                    start=True, stop=True,
                )
                dst = out_sb[:, QC * qc : QC * (qc + 1)]
                if qc < 3:
                    nc.vector.tensor_copy(out=dst, in_=fps[:])
                else:
                    nc.scalar.copy(out=dst, in_=fps[:])
                nc.sync.dma_start(out=out_d[:, QC * qc : QC * (qc + 1)], in_=dst)

    nc.finalize()
    return nc


def _host_prep(inputs):
    """Host-side layout prep: transposes, dtype casts, one-hot tables.

    Returns the list of 8 per-core input dicts for the bass kernel.
    """
    q = np.ascontiguousarray(np.asarray(inputs["query_embeddings"], dtype=np.float32))
    s = np.ascontiguousarray(np.asarray(inputs["support_embeddings"], dtype=np.float32))
    lab = np.asarray(inputs["support_labels"]).astype(np.int64)
    W1 = np.asarray(inputs["W1"], dtype=np.float32)
    b1 = np.asarray(inputs["b1"], dtype=np.float32)
    W2 = np.asarray(inputs["W2"], dtype=np.float32)

    qT = np.ascontiguousarray(q.T).astype(bf16)            # [D, Q]
    sT_full = np.ascontiguousarray(s.T).astype(np.float32) # [D, S]

    blob0 = np.zeros((128, NB), dtype=np.float32)
    for dinb in range(2):
        blob0[:, OFF_W1A + 256 * dinb : OFF_W1A + 256 * (dinb + 1)] = W1[
            128 * dinb : 128 * (dinb + 1)
        ]
        blob0[:, OFF_W1B + 256 * dinb : OFF_W1B + 256 * (dinb + 1)] = W1[
            D + 128 * dinb : D + 128 * (dinb + 1)
        ]
    for db in range(2):
        blk = W2[128 * db : 128 * (db + 1)]
        for r in range(NR):
            blob0[:, OFF_W2C + 32 * (db * NR + r) + r] = blk
        blob0[:, OFF_B1F + db] = b1[128 * db : 128 * (db + 1)]

    in_maps = []
    for c in range(N_CORES):
        lo = c * SP
        blob = blob0.copy()
        for dinb in range(2):
            blob[:, OFF_ST + SP * dinb : OFF_ST + SP * (dinb + 1)] = sT_full[
                128 * dinb : 128 * (dinb + 1), lo : lo + SP
            ]
        for sl in range(SP):
            row = 32 * (sl % 4) + sl // 4
            blob[row, OFF_OHM + lab[lo + sl]] = 1.0
            blob[row, OFF_OHM + NWAY] = 1.0
        in_maps.append({"blob": blob.astype(bf16), "qT": qT})
    return in_maps


def _combine(parts):
    """Sum per-core partials (bf16 on wire) and normalize -> [Q, NWAY] f32."""
    total = np.zeros((NWAY + 1, Q), dtype=np.float32)
    for p in parts:
        total += np.asarray(p, dtype=np.float32)
    return np.ascontiguousarray((total[:NWAY] / total[NWAY : NWAY + 1]).T)


def get_nc():
    global _compiled
    if _compiled is None:
        _compiled = _build_nc()
    return _compiled


def kernel(**inputs) -> np.ndarray:
    from concourse.bass_utils import run_bass_kernel_spmd

    nc = get_nc()
    in_maps = _host_prep(inputs)
    res = run_bass_kernel_spmd(nc, in_maps, list(range(N_CORES)))
    return _combine([res.results[c]["part"] for c in range(N_CORES)])


# revision 30
# speedup vs baseline: 1.0288x; 1.0288x over previous
"""Trainium2 Bass kernel for a Matching Network attention head (v6).

Reference computation:
    q_proj = query @ W1[:D]                       # [Q, D]
    s_proj = support @ W1[D:]                     # [S, D]
    hidden = relu(q_proj[:,None,:] + s_proj[None,:,:] + b1)   # [Q, S, D]
    scores = einsum('qsd,d->qs', hidden, W2) + b2
    weights = softmax(scores, axis=1)
    logits  = weights @ onehot(support_labels)    # [Q, n_way]

Sharding (8 cores): shard the SUPPORT set (40 rows/core), replicate
queries.  Each core emits unnormalized softmax partials
    part[w, q]  = sum_{s in shard} exp(score[s,q]) * onehot[s,w]
    part[20, q] = sum_{s in shard} exp(score[s,q])
summed and divided on the host (b2 cancels in the softmax; exp without
max-subtraction is safe: scores ~ N(0,1)).

Measured model driving the v6 schedule (see trace notes in repo memory):
  - main loop is engine-saturated at ~40us: 60 DVE relu tiles
    (tensor_scalar ADD,MAX bf16 4x: ~615ns effective) + 20 ACT relu
    tiles (~1950ns) — the wins are startup and tail latency.
  - DMA: each entry sustains only ~60-95GB/s; sync ring aggregates
    ~340GB/s over concurrent entries, scalar ring caps ~90GB/s total.
    So the startup-critical bytes ride sync as MANY SMALL entries in
    dependency-priority order and the leftovers ride scalar.
  - PE HAM: ~3.4us of sustained activity un-throttles 1.2->2.4GHz;
    idle windows re-throttle.  Warmup dummies bridge the DMA wait, and
    tail warm-keepers (dependency-pinned to late H tiles) keep the
    last-round matmuls warm.
  - startup: qpT psum->sbuf copies interleave with round-0 chunked H
    ops per q-chunk so the DVE queue never head-of-line blocks.
  - tail: only the last round's db0 half remains at the end (db1 was
    pulled into round 8), produced in two half-width chunks so the
    first exp fires ~2us before the last H op; per-qc
    exp -> final matmul -> copy -> DMA, copies on DVE (idle), last on
    ACT, DMAs on the idle sync ring.
"""

import numpy as np
import ml_dtypes

bf16 = ml_dtypes.bfloat16

N_CORES = 8
Q, D, S, NWAY = 2048, 256, 320, 20
SP = S // N_CORES          # 40 support rows per core
NQC = 4                    # q chunks of 512 (one psum bank each)
QC = Q // NQC
NR = SP // 4               # 10 rounds of 4 concurrent s-values

# const-blob column layout (bf16, [128, NB])
OFF_W1B = 0                # [128, 256] x2 (din block major)
OFF_ST = 512               # [128, 40] x2
OFF_B1F = 592              # [128, 2]: col db = b1[128*db:128*(db+1)]
OFF_W1A = 594              # [128, 256] x2
END_PA = 1106              # end of critical blob piece
OFF_W2C = 1106             # [128, 640]: 2 dblk x 10 rounds x [128, 32]
OFF_OHM = 1746             # [128, 21] x2 (ohmA | ohmB)
NB = 1788

N_WARM_PRE = 10            # PE warmup matmuls bridging the DMA wait

_compiled = None


def _act_tile_set():
    """(j, db) -> round assignment of H tiles to the ACT engine.

    18 of the 72 round-0..8 tiles go to ACT, 2 per round (the last
    round's remaining db0 half is always DVE)."""
    act = set()
    ts_idx = 0
    for r in range(NR - 1):
        for j in range(4):
            for db in range(2):
                if (ts_idx * 18) % 72 < 18:
                    act.add((r, j, db))
                ts_idx += 1
    return act


def _build_nc():
    import concourse.tile as tile
    from concourse import mybir
    from concourse.bacc import Bacc

    f32 = mybir.dt.float32
    b16 = mybir.dt.bfloat16
    RELU = mybir.ActivationFunctionType.Relu
    EXP = mybir.ActivationFunctionType.Exp
    IDENT = mybir.ActivationFunctionType.Identity
    ADD = mybir.AluOpType.add
    MAX = mybir.AluOpType.max

    nc = Bacc()
    blob_d = nc.declare_dram_parameter("blob", [128, NB], b16, isOutput=False)
    qT_d = nc.declare_dram_parameter("qT", [D, Q], b16, isOutput=False)
    out_d = nc.declare_dram_parameter("part", [NWAY + 1, Q], b16, isOutput=True)

    ACT_SET = _act_tile_set()

    with tile.TileContext(nc) as tc:
        with (
            tc.tile_pool(name="const", bufs=1) as cpool,
            tc.tile_pool(name="stage", bufs=1) as spool,
            tc.tile_pool(name="hpool", bufs=16) as hpool,
            tc.tile_pool(name="psum", bufs=8, space="PSUM") as ppool,
        ):
            # ---- inputs ----------------------------------------------
            blob_t = cpool.tile([128, NB], b16, name="blobt")
            qT_t = [spool.tile([128, Q], b16, name=f"qTt{i}") for i in range(2)]
            scratch_t = cpool.tile([128, 256], b16, name="scratch")
            b1f_t = cpool.tile([128, 2], f32, name="b1f")

            # sync ring: critical pieces, small entries, priority order
            nc.sync.dma_start(out=blob_t[:, 0:OFF_W1A], in_=blob_d[:, 0:OFF_W1A])
            nc.sync.dma_start(
                out=blob_t[:, OFF_W1A:END_PA], in_=blob_d[:, OFF_W1A:END_PA]
            )
            for qc in range(3):
                for i in range(2):
                    nc.sync.dma_start(
                        out=qT_t[i][:, QC * qc : QC * (qc + 1)],
                        in_=qT_d[128 * i : 128 * (i + 1), QC * qc : QC * (qc + 1)],
                    )
            # scalar ring (slow): W2C/ohm + the last q-chunk
            nc.scalar.dma_start(out=blob_t[:, END_PA:NB], in_=blob_d[:, END_PA:NB])
            for i in range(2):
                nc.scalar.dma_start(
                    out=qT_t[i][:, QC * 3 : Q],
                    in_=qT_d[128 * i : 128 * (i + 1), QC * 3 : Q],
                )

            def w1a(dinb, doutb):
                o = OFF_W1A + 256 * dinb + 128 * doutb
                return blob_t[:, o : o + 128]

            def w1b(dinb, doutb):
                o = OFF_W1B + 256 * dinb + 128 * doutb
                return blob_t[:, o : o + 128]

            def sT(dinb):
                o = OFF_ST + SP * dinb
                return blob_t[:, o : o + SP]

            def w2col(db, r):
                o = OFF_W2C + 32 * (db * NR + r)
                return blob_t[:, o : o + 32]

            def ohm(half):
                o = OFF_OHM + (NWAY + 1) * half
                return blob_t[:, o : o + NWAY + 1]

            # ---- PE warmup (no DMA deps): un-throttle HAM early ------
            nc.vector.memset(scratch_t[:], 0.0)
            warm_ps = [
                ppool.tile([128, 256], f32, tag="ps", name=f"warmps{i}")
                for i in range(2)
            ]
            for i in range(N_WARM_PRE):
                nc.tensor.matmul(
                    warm_ps[i % 2][:], scratch_t[:, 0:128], scratch_t[:],
                    start=True, stop=True,
                )

            # b1 (bf16 cols in blob) -> fp32 for activation bias
            nc.vector.tensor_copy(out=b1f_t[:], in_=blob_t[:, OFF_B1F : OFF_B1F + 2])

            # ---- qpT production interleaved with round-0 H chunks ----
            # qc0's qpT is emitted BEFORE spb (its DMA lands first; spb
            # waiting on the later blob piece must not head-of-line
            # block it on the PE queue).  DVE queue: cast(qc0) -> r0
            # qc0 chunks -> r0 qc1-3.  All other qpT copies ride ACT.
            qpT_t = [spool.tile([128, Q], b16, name=f"qpT{i}") for i in range(2)]
            spb_t = [cpool.tile([128, SP], f32, name=f"spb{i}") for i in range(2)]
            r0_dve = [(j, db) for j in range(4) for db in range(2)
                      if (0, j, db) not in ACT_SET]
            h0_tiles = {}
            for (j, db) in r0_dve:
                h0_tiles[(j, db)] = hpool.tile(
                    [128, Q], b16, tag="Hd", bufs=26, name=f"h{j}_{db}"
                )

            def emit_r0_chunk(lo, hi):
                for (j, db) in r0_dve:
                    nc.vector.tensor_scalar(
                        out=h0_tiles[(j, db)][:, lo:hi],
                        in0=qpT_t[db][:, lo:hi],
                        scalar1=spb_t[db][:, j : j + 1],
                        scalar2=0.0, op0=ADD, op1=MAX,
                    )

            def emit_qpT(qc):
                for db in range(2):
                    qps = ppool.tile([128, QC], f32, tag="ps", name=f"qps{db}{qc}")
                    nc.tensor.matmul(
                        qps[:], w1a(0, db), qT_t[0][:, QC * qc : QC * (qc + 1)],
                        start=True, stop=False,
                    )
                    nc.tensor.matmul(
                        qps[:], w1a(1, db), qT_t[1][:, QC * qc : QC * (qc + 1)],
                        start=False, stop=True,
                    )
                    dst = qpT_t[db][:, QC * qc : QC * (qc + 1)]
                    if qc == 0 and db == 0:
                        nc.vector.tensor_copy(out=dst, in_=qps[:])
                    else:
                        nc.scalar.copy(out=dst, in_=qps[:])

            emit_qpT(0)

            # ---- spbT = W1b^T @ supportT (+b1 on the copy)  [2][128,SP]
            for db in range(2):
                sps = ppool.tile([128, QC], f32, tag="ps", name=f"sps{db}")
                nc.tensor.matmul(sps[:, :SP], w1b(0, db), sT(0), start=True, stop=False)
                nc.tensor.matmul(sps[:, :SP], w1b(1, db), sT(1), start=False, stop=True)
                nc.scalar.activation(
                    spb_t[db][:], sps[:, :SP], IDENT, bias=b1f_t[:, db : db + 1]
                )

            emit_r0_chunk(0, QC)
            for qc in range(1, NQC):
                emit_qpT(qc)
            emit_r0_chunk(QC, 2 * QC)
            emit_r0_chunk(2 * QC, Q)

            # ---- main loop -------------------------------------------
            e_t = spool.tile([128, Q], b16, name="et")
            out_sb = spool.tile([NWAY + 1, Q], b16, name="outsb")
            scores_ps = [
                ppool.tile([128, QC], f32, tag="ps", name=f"sc{qc}")
                for qc in range(NQC)
            ]

            for r in range(NR - 1):
                h_tiles = {}
                for j in range(4):
                    sl = 4 * r + j
                    for db in range(2):
                        if r == 0:
                            h = h0_tiles[(j, db)] if (j, db) in r0_dve else None
                        else:
                            h = None
                        if h is None:
                            if (r, j, db) in ACT_SET:
                                h = hpool.tile(
                                    [128, Q], b16, tag="Ha", bufs=8, name=f"h{sl}_{db}"
                                )
                                nc.scalar.activation(
                                    h[:], qpT_t[db][:], RELU,
                                    bias=spb_t[db][:, sl : sl + 1],
                                )
                            else:
                                h = hpool.tile(
                                    [128, Q], b16, tag="Hd", bufs=26, name=f"h{sl}_{db}"
                                )
                                nc.vector.tensor_scalar(
                                    out=h[:], in0=qpT_t[db][:],
                                    scalar1=spb_t[db][:, sl : sl + 1],
                                    scalar2=0.0, op0=ADD, op1=MAX,
                                )
                        h_tiles[(j, db)] = h
                for db in range(2):
                    for qc in range(NQC):
                        for j in range(4):
                            nc.tensor.matmul(
                                scores_ps[qc][32 * j : 32 * j + 32, :],
                                w2col(db, r),
                                h_tiles[(j, db)][:, QC * qc : QC * (qc + 1)],
                                start=(r == 0 and db == 0),
                                stop=False,
                                tile_position=(0, 32 * j),
                                skip_group_check=True,
                            )
                if r == NR - 2:
                    # pull the LAST round's db1 half forward so only 4
                    # H tiles + 4 matmul groups gate the tail.
                    h9 = []
                    for j in range(4):
                        sl = 4 * (NR - 1) + j
                        h = hpool.tile(
                            [128, Q], b16, tag="Hd", bufs=26, name=f"h{sl}_1"
                        )
                        nc.vector.tensor_scalar(
                            out=h[:], in0=qpT_t[1][:],
                            scalar1=spb_t[1][:, sl : sl + 1],
                            scalar2=0.0, op0=ADD, op1=MAX,
                        )
                        h9.append(h)
                    for qc in range(NQC):
                        for j in range(4):
                            nc.tensor.matmul(
                                scores_ps[qc][32 * j : 32 * j + 32, :],
                                w2col(1, NR - 1),
                                h9[j][:, QC * qc : QC * (qc + 1)],
                                start=False,
                                stop=False,
                                tile_position=(0, 32 * j),
                                skip_group_check=True,
                            )

            # ---- last round (db0 only) + per-qc tail -----------------
            # H in three chunks [qc01 | qc2 | qc3] so qc0-qc2's scores
            # close (and their exps start) before the last H columns.
            r = NR - 1
            hl_tiles = {}
            for j in range(3):
                hl_tiles[j] = hpool.tile(
                    [128, Q], b16, tag="Hd", bufs=26, name=f"h{4 * r + j}_0"
                )
            # j=3 rides ACT full-width: ACT's relu stream drains ~6us
            # before DVE's, so this tile is ready before its matmuls,
            # and DVE's tail-gating chunk stream shrinks by a quarter.
            hl_tiles[3] = hpool.tile(
                [128, Q], b16, tag="Ha", bufs=8, name=f"h{4 * r + 3}_0"
            )
            nc.scalar.activation(
                hl_tiles[3][:], qpT_t[0][:], RELU,
                bias=spb_t[0][:, 4 * r + 3 : 4 * r + 4],
            )
            for (lo, hi) in ((0, 2 * QC), (2 * QC, 3 * QC), (3 * QC, Q)):
                for j in range(3):
                    sl = 4 * r + j
                    nc.vector.tensor_scalar(
                        out=hl_tiles[j][:, lo:hi], in0=qpT_t[0][:, lo:hi],
                        scalar1=spb_t[0][:, sl : sl + 1],
                        scalar2=0.0, op0=ADD, op1=MAX,
                    )
                if lo == 0:
                    # PE warm-keepers pinned to the first chunks (can't
                    # be hoisted): keep HAM at 8/8 into the tail.
                    for i in range(2):
                        wp = ppool.tile([128, 256], f32, tag="ps", name=f"tw{i}")
                        nc.tensor.matmul(
                            wp[:],
                            hl_tiles[2 * i][:, 0:128],
                            hl_tiles[2 * i][:, 0:256],
                            start=True, stop=True,
                        )
            for qc in range(NQC):
                for j in range(4):
                    nc.tensor.matmul(
                        scores_ps[qc][32 * j : 32 * j + 32, :],
                        w2col(0, r),
                        hl_tiles[j][:, QC * qc : QC * (qc + 1)],
                        start=False,
                        stop=(j == 3),
                        tile_position=(0, 32 * j),
                        skip_group_check=True,
                    )
                nc.scalar.activation(
                    e_t[:, QC * qc : QC * (qc + 1)], scores_ps[qc][:], EXP,
                )
                fps = ppool.tile([NWAY + 1, QC], f32, tag="ps", name=f"fps{qc}")
                nc.tensor.matmul(
                    fps[:], ohm(0), e_t[:, QC * qc : QC * (qc + 1)],
                    start=True, stop=True,
                )
                dst = out_sb[:, QC * qc : QC * (qc + 1)]
                if qc < 3:
                    nc.vector.tensor_copy(out=dst, in_=fps[:])
                else:
                    nc.scalar.copy(out=dst, in_=fps[:])
                nc.sync.dma_start(out=out_d[:, QC * qc : QC * (qc + 1)], in_=dst)

    nc.finalize()
    return nc


def _host_prep(inputs):
    """Host-side layout prep: transposes, dtype casts, one-hot tables.

    Returns the list of 8 per-core input dicts for the bass kernel.
    """
    q = np.ascontiguousarray(np.asarray(inputs["query_embeddings"], dtype=np.float32))
    s = np.ascontiguousarray(np.asarray(inputs["support_embeddings"], dtype=np.float32))
    lab = np.asarray(inputs["support_labels"]).astype(np.int64)
    W1 = np.asarray(inputs["W1"], dtype=np.float32)
    b1 = np.asarray(inputs["b1"], dtype=np.float32)
    W2 = np.asarray(inputs["W2"], dtype=np.float32)

    qT = np.ascontiguousarray(q.T).astype(bf16)            # [D, Q]
    sT_full = np.ascontiguousarray(s.T).astype(np.float32) # [D, S]

    blob0 = np.zeros((128, NB), dtype=np.float32)
    for dinb in range(2):
        blob0[:, OFF_W1A + 256 * dinb : OFF_W1A + 256 * (dinb + 1)] = W1[
            128 * dinb : 128 * (dinb + 1)
        ]
        blob0[:, OFF_W1B + 256 * dinb : OFF_W1B + 256 * (dinb + 1)] = W1[
            D + 128 * dinb : D + 128 * (dinb + 1)
        ]
    for db in range(2):
        blk = W2[128 * db : 128 * (db + 1)]
        for r in range(NR):
            blob0[:, OFF_W2C + 32 * (db * NR + r) + r] = blk
        blob0[:, OFF_B1F + db] = b1[128 * db : 128 * (db + 1)]

    in_maps = []
    for c in range(N_CORES):
        lo = c * SP
        blob = blob0.copy()
        for dinb in range(2):
            blob[:, OFF_ST + SP * dinb : OFF_ST + SP * (dinb + 1)] = sT_full[
                128 * dinb : 128 * (dinb + 1), lo : lo + SP
            ]
        for sl in range(SP):
            row = 32 * (sl % 4) + sl // 4
            blob[row, OFF_OHM + lab[lo + sl]] = 1.0
            blob[row, OFF_OHM + NWAY] = 1.0
        in_maps.append({"blob": blob.astype(bf16), "qT": qT})
    return in_maps


def _combine(parts):
    """Sum per-core partials (bf16 on wire) and normalize -> [Q, NWAY] f32."""
    total = np.zeros((NWAY + 1, Q), dtype=np.float32)
    for p in parts:
        total += np.asarray(p, dtype=np.float32)
    return np.ascontiguousarray((total[:NWAY] / total[NWAY : NWAY + 1]).T)


def get_nc():
    global _compiled
    if _compiled is None:
        _compiled = _build_nc()
    return _compiled


def kernel(**inputs) -> np.ndarray:
    from concourse.bass_utils import run_bass_kernel_spmd

    nc = get_nc()
    in_maps = _host_prep(inputs)
    res = run_bass_kernel_spmd(nc, in_maps, list(range(N_CORES)))
    return _combine([res.results[c]["part"] for c in range(N_CORES)])


# revision 31
# speedup vs baseline: 1.0552x; 1.0256x over previous
"""Trainium2 Bass kernel for a Matching Network attention head (v6).

Reference computation:
    q_proj = query @ W1[:D]                       # [Q, D]
    s_proj = support @ W1[D:]                     # [S, D]
    hidden = relu(q_proj[:,None,:] + s_proj[None,:,:] + b1)   # [Q, S, D]
    scores = einsum('qsd,d->qs', hidden, W2) + b2
    weights = softmax(scores, axis=1)
    logits  = weights @ onehot(support_labels)    # [Q, n_way]

Sharding (8 cores): shard the SUPPORT set (40 rows/core), replicate
queries.  Each core emits unnormalized softmax partials
    part[w, q]  = sum_{s in shard} exp(score[s,q]) * onehot[s,w]
    part[20, q] = sum_{s in shard} exp(score[s,q])
summed and divided on the host (b2 cancels in the softmax; exp without
max-subtraction is safe: scores ~ N(0,1)).

Measured model driving the v6 schedule (see trace notes in repo memory):
  - main loop is engine-saturated at ~40us: 60 DVE relu tiles
    (tensor_scalar ADD,MAX bf16 4x: ~615ns effective) + 20 ACT relu
    tiles (~1950ns) — the wins are startup and tail latency.
  - DMA: each entry sustains only ~60-95GB/s; sync ring aggregates
    ~340GB/s over concurrent entries, scalar ring caps ~90GB/s total.
    So the startup-critical bytes ride sync as MANY SMALL entries in
    dependency-priority order and the leftovers ride scalar.
  - PE HAM: ~3.4us of sustained activity un-throttles 1.2->2.4GHz;
    idle windows re-throttle.  Warmup dummies bridge the DMA wait, and
    tail warm-keepers (dependency-pinned to late H tiles) keep the
    last-round matmuls warm.
  - startup: qpT psum->sbuf copies interleave with round-0 chunked H
    ops per q-chunk so the DVE queue never head-of-line blocks.
  - tail: only the last round's db0 half remains at the end (db1 was
    pulled into round 8), produced in two half-width chunks so the
    first exp fires ~2us before the last H op; per-qc
    exp -> final matmul -> copy -> DMA, copies on DVE (idle), last on
    ACT, DMAs on the idle sync ring.
"""

import numpy as np
import ml_dtypes

bf16 = ml_dtypes.bfloat16

N_CORES = 8
Q, D, S, NWAY = 2048, 256, 320, 20
SP = S // N_CORES          # 40 support rows per core
NQC = 4                    # q chunks of 512 (one psum bank each)
QC = Q // NQC
NR = SP // 4               # 10 rounds of 4 concurrent s-values

# const-blob column layout (bf16, [128, NB])
OFF_W1B = 0                # [128, 256] x2 (din block major)
OFF_ST = 512               # [128, 40] x2
OFF_B1F = 592              # [128, 2]: col db = b1[128*db:128*(db+1)]
OFF_W1A = 594              # [128, 256] x2
END_PA = 1106              # end of critical blob piece
OFF_W2C = 1106             # [128, 640]: 2 dblk x 10 rounds x [128, 32]
OFF_OHM = 1746             # [128, 21] x2 (ohmA | ohmB)
NB = 1788

N_WARM_PRE = 10            # PE warmup matmuls bridging the DMA wait

_compiled = None


def _act_tile_set():
    """(j, db) -> round assignment of H tiles to the ACT engine.

    18 of the 72 round-0..8 tiles go to ACT, 2 per round (the last
    round's remaining db0 half is always DVE)."""
    act = set()
    ts_idx = 0
    for r in range(NR - 1):
        for j in range(4):
            for db in range(2):
                if (ts_idx * 18) % 72 < 18:
                    act.add((r, j, db))
                ts_idx += 1
    return act


def _build_nc():
    import concourse.tile as tile
    from concourse import mybir
    from concourse.bacc import Bacc

    f32 = mybir.dt.float32
    b16 = mybir.dt.bfloat16
    RELU = mybir.ActivationFunctionType.Relu
    EXP = mybir.ActivationFunctionType.Exp
    IDENT = mybir.ActivationFunctionType.Identity
    ADD = mybir.AluOpType.add
    MAX = mybir.AluOpType.max

    nc = Bacc()
    blob_d = nc.declare_dram_parameter("blob", [128, NB], b16, isOutput=False)
    qT_d = nc.declare_dram_parameter("qT", [D, Q], b16, isOutput=False)
    out_d = nc.declare_dram_parameter("part", [NWAY + 1, Q], b16, isOutput=True)

    ACT_SET = _act_tile_set()

    with tile.TileContext(nc) as tc:
        with (
            tc.tile_pool(name="const", bufs=1) as cpool,
            tc.tile_pool(name="stage", bufs=1) as spool,
            tc.tile_pool(name="hpool", bufs=16) as hpool,
            tc.tile_pool(name="psum", bufs=8, space="PSUM") as ppool,
        ):
            # ---- inputs ----------------------------------------------
            blob_t = cpool.tile([128, NB], b16, name="blobt")
            qT_t = [spool.tile([128, Q], b16, name=f"qTt{i}") for i in range(2)]
            scratch_t = cpool.tile([128, 512], b16, name="scratch")
            b1f_t = cpool.tile([128, 2], f32, name="b1f")

            # sync ring: critical pieces, small entries, priority order
            nc.sync.dma_start(out=blob_t[:, 0:OFF_W1A], in_=blob_d[:, 0:OFF_W1A])
            nc.sync.dma_start(
                out=blob_t[:, OFF_W1A:END_PA], in_=blob_d[:, OFF_W1A:END_PA]
            )
            for qc in range(3):
                for i in range(2):
                    nc.sync.dma_start(
                        out=qT_t[i][:, QC * qc : QC * (qc + 1)],
                        in_=qT_d[128 * i : 128 * (i + 1), QC * qc : QC * (qc + 1)],
                    )
            # scalar ring (slow): W2C/ohm + the last q-chunk
            nc.scalar.dma_start(out=blob_t[:, END_PA:NB], in_=blob_d[:, END_PA:NB])
            for i in range(2):
                nc.scalar.dma_start(
                    out=qT_t[i][:, QC * 3 : Q],
                    in_=qT_d[128 * i : 128 * (i + 1), QC * 3 : Q],
                )

            def w1a(dinb, doutb):
                o = OFF_W1A + 256 * dinb + 128 * doutb
                return blob_t[:, o : o + 128]

            def w1b(dinb, doutb):
                o = OFF_W1B + 256 * dinb + 128 * doutb
                return blob_t[:, o : o + 128]

            def sT(dinb):
                o = OFF_ST + SP * dinb
                return blob_t[:, o : o + SP]

            def w2col(db, r):
                o = OFF_W2C + 32 * (db * NR + r)
                return blob_t[:, o : o + 32]

            def ohm(half):
                o = OFF_OHM + (NWAY + 1) * half
                return blob_t[:, o : o + NWAY + 1]

            # ---- PE warmup (no DMA deps): un-throttle HAM early ------
            nc.vector.memset(scratch_t[:], 0.0)
            warm_ps = [
                ppool.tile([128, 512], f32, tag="ps", name=f"warmps{i}")
                for i in range(2)
            ]
            for i in range(N_WARM_PRE):
                nc.tensor.matmul(
                    warm_ps[i % 2][:], scratch_t[:, 0:128], scratch_t[:],
                    start=True, stop=True,
                )

            # b1 (bf16 cols in blob) -> fp32 for activation bias
            nc.vector.tensor_copy(out=b1f_t[:], in_=blob_t[:, OFF_B1F : OFF_B1F + 2])

            # ---- qpT production interleaved with round-0 H chunks ----
            # qc0's qpT is emitted BEFORE spb (its DMA lands first; spb
            # waiting on the later blob piece must not head-of-line
            # block it on the PE queue).  DVE queue: cast(qc0) -> r0
            # qc0 chunks -> r0 qc1-3.  All other qpT copies ride ACT.
            qpT_t = [spool.tile([128, Q], b16, name=f"qpT{i}") for i in range(2)]
            spb_t = [cpool.tile([128, SP], f32, name=f"spb{i}") for i in range(2)]
            r0_dve = [(j, db) for j in range(4) for db in range(2)
                      if (0, j, db) not in ACT_SET]
            h0_tiles = {}
            for (j, db) in r0_dve:
                h0_tiles[(j, db)] = hpool.tile(
                    [128, Q], b16, tag="Hd", bufs=26, name=f"h{j}_{db}"
                )

            def emit_r0_chunk(lo, hi):
                for (j, db) in r0_dve:
                    nc.vector.tensor_scalar(
                        out=h0_tiles[(j, db)][:, lo:hi],
                        in0=qpT_t[db][:, lo:hi],
                        scalar1=spb_t[db][:, j : j + 1],
                        scalar2=0.0, op0=ADD, op1=MAX,
                    )

            def emit_qpT(qc):
                for db in range(2):
                    qps = ppool.tile([128, QC], f32, tag="ps", name=f"qps{db}{qc}")
                    nc.tensor.matmul(
                        qps[:], w1a(0, db), qT_t[0][:, QC * qc : QC * (qc + 1)],
                        start=True, stop=False,
                    )
                    nc.tensor.matmul(
                        qps[:], w1a(1, db), qT_t[1][:, QC * qc : QC * (qc + 1)],
                        start=False, stop=True,
                    )
                    dst = qpT_t[db][:, QC * qc : QC * (qc + 1)]
                    if qc == 0 and db == 0:
                        nc.vector.tensor_copy(out=dst, in_=qps[:])
                    else:
                        nc.scalar.copy(out=dst, in_=qps[:])

            emit_qpT(0)

            # ---- spbT = W1b^T @ supportT (+b1 on the copy)  [2][128,SP]
            for db in range(2):
                sps = ppool.tile([128, QC], f32, tag="ps", name=f"sps{db}")
                nc.tensor.matmul(sps[:, :SP], w1b(0, db), sT(0), start=True, stop=False)
                nc.tensor.matmul(sps[:, :SP], w1b(1, db), sT(1), start=False, stop=True)
                nc.scalar.activation(
                    spb_t[db][:], sps[:, :SP], IDENT, bias=b1f_t[:, db : db + 1]
                )

            emit_r0_chunk(0, QC)
            for qc in range(1, NQC):
                emit_qpT(qc)
            emit_r0_chunk(QC, 2 * QC)
            emit_r0_chunk(2 * QC, Q)

            # ---- main loop -------------------------------------------
            e_t = spool.tile([128, Q], b16, name="et")
            out_sb = spool.tile([NWAY + 1, Q], b16, name="outsb")
            scores_ps = [
                ppool.tile([128, QC], f32, tag="ps", name=f"sc{qc}")
                for qc in range(NQC)
            ]

            for r in range(NR - 1):
                h_tiles = {}
                for j in range(4):
                    sl = 4 * r + j
                    for db in range(2):
                        if r == 0:
                            h = h0_tiles[(j, db)] if (j, db) in r0_dve else None
                        else:
                            h = None
                        if h is None:
                            if (r, j, db) in ACT_SET:
                                h = hpool.tile(
                                    [128, Q], b16, tag="Ha", bufs=8, name=f"h{sl}_{db}"
                                )
                                nc.scalar.activation(
                                    h[:], qpT_t[db][:], RELU,
                                    bias=spb_t[db][:, sl : sl + 1],
                                )
                            else:
                                h = hpool.tile(
                                    [128, Q], b16, tag="Hd", bufs=26, name=f"h{sl}_{db}"
                                )
                                nc.vector.tensor_scalar(
                                    out=h[:], in0=qpT_t[db][:],
                                    scalar1=spb_t[db][:, sl : sl + 1],
                                    scalar2=0.0, op0=ADD, op1=MAX,
                                )
                        h_tiles[(j, db)] = h
                for db in range(2):
                    for qc in range(NQC):
                        for j in range(4):
                            nc.tensor.matmul(
                                scores_ps[qc][32 * j : 32 * j + 32, :],
                                w2col(db, r),
                                h_tiles[(j, db)][:, QC * qc : QC * (qc + 1)],
                                start=(r == 0 and db == 0),
                                stop=False,
                                tile_position=(0, 32 * j),
                                skip_group_check=True,
                            )
                if r == NR - 2:
                    # pull the LAST round's db1 half forward so only 4
                    # H tiles + 4 matmul groups gate the tail.
                    h9 = []
                    for j in range(4):
                        sl = 4 * (NR - 1) + j
                        h = hpool.tile(
                            [128, Q], b16, tag="Hd", bufs=26, name=f"h{sl}_1"
                        )
                        nc.vector.tensor_scalar(
                            out=h[:], in0=qpT_t[1][:],
                            scalar1=spb_t[1][:, sl : sl + 1],
                            scalar2=0.0, op0=ADD, op1=MAX,
                        )
                        h9.append(h)
                    for qc in range(NQC):
                        for j in range(4):
                            nc.tensor.matmul(
                                scores_ps[qc][32 * j : 32 * j + 32, :],
                                w2col(1, NR - 1),
                                h9[j][:, QC * qc : QC * (qc + 1)],
                                start=False,
                                stop=False,
                                tile_position=(0, 32 * j),
                                skip_group_check=True,
                            )

            # ---- last round (db0 only) + per-qc tail -----------------
            # H in three chunks [qc01 | qc2 | qc3] so qc0-qc2's scores
            # close (and their exps start) before the last H columns.
            r = NR - 1
            hl_tiles = {}
            for j in range(3):
                hl_tiles[j] = hpool.tile(
                    [128, Q], b16, tag="Hd", bufs=26, name=f"h{4 * r + j}_0"
                )
            # j=3 rides ACT full-width: ACT's relu stream drains ~6us
            # before DVE's, so this tile is ready before its matmuls,
            # and DVE's tail-gating chunk stream shrinks by a quarter.
            hl_tiles[3] = hpool.tile(
                [128, Q], b16, tag="Ha", bufs=8, name=f"h{4 * r + 3}_0"
            )
            nc.scalar.activation(
                hl_tiles[3][:], qpT_t[0][:], RELU,
                bias=spb_t[0][:, 4 * r + 3 : 4 * r + 4],
            )
            for (lo, hi) in ((0, 2 * QC), (2 * QC, 3 * QC), (3 * QC, Q)):
                for j in range(3):
                    sl = 4 * r + j
                    nc.vector.tensor_scalar(
                        out=hl_tiles[j][:, lo:hi], in0=qpT_t[0][:, lo:hi],
                        scalar1=spb_t[0][:, sl : sl + 1],
                        scalar2=0.0, op0=ADD, op1=MAX,
                    )
                if lo == 0:
                    # PE warm-keepers pinned to the first chunks (can't
                    # be hoisted): keep HAM at 8/8 into the tail.
                    for i in range(2):
                        wp = ppool.tile([128, 256], f32, tag="ps", name=f"tw{i}")
                        nc.tensor.matmul(
                            wp[:],
                            hl_tiles[2 * i][:, 0:128],
                            hl_tiles[2 * i][:, 0:256],
                            start=True, stop=True,
                        )
            # all four score groups first (PE queue must not
            # head-of-line block later groups behind exp-gated fps
            # matmuls), then the fps/copy/DMA chains per qc.
            for qc in range(NQC):
                for j in range(4):
                    nc.tensor.matmul(
                        scores_ps[qc][32 * j : 32 * j + 32, :],
                        w2col(0, r),
                        hl_tiles[j][:, QC * qc : QC * (qc + 1)],
                        start=False,
                        stop=(j == 3),
                        tile_position=(0, 32 * j),
                        skip_group_check=True,
                    )
                nc.scalar.activation(
                    e_t[:, QC * qc : QC * (qc + 1)], scores_ps[qc][:], EXP,
                )
            for qc in range(NQC):
                fps = ppool.tile([NWAY + 1, QC], f32, tag="ps", name=f"fps{qc}")
                nc.tensor.matmul(
                    fps[:], ohm(0), e_t[:, QC * qc : QC * (qc + 1)],
                    start=True, stop=True,
                )
                dst = out_sb[:, QC * qc : QC * (qc + 1)]
                if qc < 3:
                    nc.vector.tensor_copy(out=dst, in_=fps[:])
                else:
                    nc.scalar.copy(out=dst, in_=fps[:])
                nc.sync.dma_start(out=out_d[:, QC * qc : QC * (qc + 1)], in_=dst)

    nc.finalize()
    return nc


def _host_prep(inputs):
    """Host-side layout prep: transposes, dtype casts, one-hot tables.

    Returns the list of 8 per-core input dicts for the bass kernel.
    """
    q = np.ascontiguousarray(np.asarray(inputs["query_embeddings"], dtype=np.float32))
    s = np.ascontiguousarray(np.asarray(inputs["support_embeddings"], dtype=np.float32))
    lab = np.asarray(inputs["support_labels"]).astype(np.int64)
    W1 = np.asarray(inputs["W1"], dtype=np.float32)
    b1 = np.asarray(inputs["b1"], dtype=np.float32)
    W2 = np.asarray(inputs["W2"], dtype=np.float32)

    qT = np.ascontiguousarray(q.T).astype(bf16)            # [D, Q]
    sT_full = np.ascontiguousarray(s.T).astype(np.float32) # [D, S]

    blob0 = np.zeros((128, NB), dtype=np.float32)
    for dinb in range(2):
        blob0[:, OFF_W1A + 256 * dinb : OFF_W1A + 256 * (dinb + 1)] = W1[
            128 * dinb : 128 * (dinb + 1)
        ]
        blob0[:, OFF_W1B + 256 * dinb : OFF_W1B + 256 * (dinb + 1)] = W1[
            D + 128 * dinb : D + 128 * (dinb + 1)
        ]
    for db in range(2):
        blk = W2[128 * db : 128 * (db + 1)]
        for r in range(NR):
            blob0[:, OFF_W2C + 32 * (db * NR + r) + r] = blk
        blob0[:, OFF_B1F + db] = b1[128 * db : 128 * (db + 1)]

    in_maps = []
    for c in range(N_CORES):
        lo = c * SP
        blob = blob0.copy()
        for dinb in range(2):
            blob[:, OFF_ST + SP * dinb : OFF_ST + SP * (dinb + 1)] = sT_full[
                128 * dinb : 128 * (dinb + 1), lo : lo + SP
            ]
        for sl in range(SP):
            row = 32 * (sl % 4) + sl // 4
            blob[row, OFF_OHM + lab[lo + sl]] = 1.0
            blob[row, OFF_OHM + NWAY] = 1.0
        in_maps.append({"blob": blob.astype(bf16), "qT": qT})
    return in_maps


def _combine(parts):
    """Sum per-core partials (bf16 on wire) and normalize -> [Q, NWAY] f32."""
    total = np.zeros((NWAY + 1, Q), dtype=np.float32)
    for p in parts:
        total += np.asarray(p, dtype=np.float32)
    return np.ascontiguousarray((total[:NWAY] / total[NWAY : NWAY + 1]).T)


def get_nc():
    global _compiled
    if _compiled is None:
        _compiled = _build_nc()
    return _compiled


def kernel(**inputs) -> np.ndarray:
    from concourse.bass_utils import run_bass_kernel_spmd

    nc = get_nc()
    in_maps = _host_prep(inputs)
    res = run_bass_kernel_spmd(nc, in_maps, list(range(N_CORES)))
    return _combine([res.results[c]["part"] for c in range(N_CORES)])


# revision 32
# speedup vs baseline: 1.0688x; 1.0129x over previous
"""Trainium2 Bass kernel for a Matching Network attention head (v6).

Reference computation:
    q_proj = query @ W1[:D]                       # [Q, D]
    s_proj = support @ W1[D:]                     # [S, D]
    hidden = relu(q_proj[:,None,:] + s_proj[None,:,:] + b1)   # [Q, S, D]
    scores = einsum('qsd,d->qs', hidden, W2) + b2
    weights = softmax(scores, axis=1)
    logits  = weights @ onehot(support_labels)    # [Q, n_way]

Sharding (8 cores): shard the SUPPORT set (40 rows/core), replicate
queries.  Each core emits unnormalized softmax partials
    part[w, q]  = sum_{s in shard} exp(score[s,q]) * onehot[s,w]
    part[20, q] = sum_{s in shard} exp(score[s,q])
summed and divided on the host (b2 cancels in the softmax; exp without
max-subtraction is safe: scores ~ N(0,1)).

Measured model driving the v6 schedule (see trace notes in repo memory):
  - main loop is engine-saturated at ~40us: 60 DVE relu tiles
    (tensor_scalar ADD,MAX bf16 4x: ~615ns effective) + 20 ACT relu
    tiles (~1950ns) — the wins are startup and tail latency.
  - DMA: each entry sustains only ~60-95GB/s; sync ring aggregates
    ~340GB/s over concurrent entries, scalar ring caps ~90GB/s total.
    So the startup-critical bytes ride sync as MANY SMALL entries in
    dependency-priority order and the leftovers ride scalar.
  - PE HAM: ~3.4us of sustained activity un-throttles 1.2->2.4GHz;
    idle windows re-throttle.  Warmup dummies bridge the DMA wait, and
    tail warm-keepers (dependency-pinned to late H tiles) keep the
    last-round matmuls warm.
  - startup: qpT psum->sbuf copies interleave with round-0 chunked H
    ops per q-chunk so the DVE queue never head-of-line blocks.
  - tail: only the last round's db0 half remains at the end (db1 was
    pulled into round 8), produced in two half-width chunks so the
    first exp fires ~2us before the last H op; per-qc
    exp -> final matmul -> copy -> DMA, copies on DVE (idle), last on
    ACT, DMAs on the idle sync ring.
"""

import numpy as np
import ml_dtypes

bf16 = ml_dtypes.bfloat16

N_CORES = 8
Q, D, S, NWAY = 2048, 256, 320, 20
SP = S // N_CORES          # 40 support rows per core
NQC = 4                    # q chunks of 512 (one psum bank each)
QC = Q // NQC
NR = SP // 4               # 10 rounds of 4 concurrent s-values

# const-blob column layout (bf16, [128, NB])
OFF_W1B = 0                # [128, 256] x2 (din block major)
OFF_ST = 512               # [128, 40] x2
OFF_B1F = 592              # [128, 2]: col db = b1[128*db:128*(db+1)]
OFF_W1A = 594              # [128, 256] x2
END_PA = 1106              # end of critical blob piece
OFF_W2C = 1106             # [128, 640]: 2 dblk x 10 rounds x [128, 32]
OFF_OHM = 1746             # [128, 21] x2 (ohmA | ohmB)
NB = 1788

N_WARM_PRE = 10            # PE warmup matmuls bridging the DMA wait

_compiled = None


def _act_tile_set():
    """(j, db) -> round assignment of H tiles to the ACT engine.

    18 of the 72 round-0..8 tiles go to ACT, 2 per round (the last
    round's remaining db0 half is always DVE)."""
    act = set()
    ts_idx = 0
    for r in range(NR - 1):
        for j in range(4):
            for db in range(2):
                if (ts_idx * 18) % 72 < 18:
                    act.add((r, j, db))
                ts_idx += 1
    return act


def _build_nc():
    import concourse.tile as tile
    from concourse import mybir
    from concourse.bacc import Bacc

    f32 = mybir.dt.float32
    b16 = mybir.dt.bfloat16
    RELU = mybir.ActivationFunctionType.Relu
    EXP = mybir.ActivationFunctionType.Exp
    IDENT = mybir.ActivationFunctionType.Identity
    ADD = mybir.AluOpType.add
    MAX = mybir.AluOpType.max

    nc = Bacc()
    blob_d = nc.declare_dram_parameter("blob", [128, NB], b16, isOutput=False)
    qT_d = nc.declare_dram_parameter("qT", [D, Q], b16, isOutput=False)
    out_d = nc.declare_dram_parameter("part", [NWAY + 1, Q], b16, isOutput=True)

    ACT_SET = _act_tile_set()

    with tile.TileContext(nc) as tc:
        with (
            tc.tile_pool(name="const", bufs=1) as cpool,
            tc.tile_pool(name="stage", bufs=1) as spool,
            tc.tile_pool(name="hpool", bufs=16) as hpool,
            tc.tile_pool(name="psum", bufs=8, space="PSUM") as ppool,
        ):
            # ---- inputs ----------------------------------------------
            blob_t = cpool.tile([128, NB], b16, name="blobt")
            qT_t = [spool.tile([128, Q], b16, name=f"qTt{i}") for i in range(2)]
            scratch_t = cpool.tile([128, 512], b16, name="scratch")
            b1f_t = cpool.tile([128, 2], f32, name="b1f")

            # sync ring: critical pieces, small entries, priority order
            nc.sync.dma_start(out=blob_t[:, 0:OFF_W1A], in_=blob_d[:, 0:OFF_W1A])
            nc.sync.dma_start(
                out=blob_t[:, OFF_W1A:END_PA], in_=blob_d[:, OFF_W1A:END_PA]
            )
            for qc in range(3):
                for i in range(2):
                    nc.sync.dma_start(
                        out=qT_t[i][:, QC * qc : QC * (qc + 1)],
                        in_=qT_d[128 * i : 128 * (i + 1), QC * qc : QC * (qc + 1)],
                    )
            # scalar ring (slow): W2C/ohm + the last q-chunk
            nc.scalar.dma_start(out=blob_t[:, END_PA:NB], in_=blob_d[:, END_PA:NB])
            for i in range(2):
                nc.scalar.dma_start(
                    out=qT_t[i][:, QC * 3 : Q],
                    in_=qT_d[128 * i : 128 * (i + 1), QC * 3 : Q],
                )

            def w1a(dinb, doutb):
                o = OFF_W1A + 256 * dinb + 128 * doutb
                return blob_t[:, o : o + 128]

            def w1b(dinb, doutb):
                o = OFF_W1B + 256 * dinb + 128 * doutb
                return blob_t[:, o : o + 128]

            def sT(dinb):
                o = OFF_ST + SP * dinb
                return blob_t[:, o : o + SP]

            def w2col(db, r):
                o = OFF_W2C + 32 * (db * NR + r)
                return blob_t[:, o : o + 32]

            def ohm(half):
                o = OFF_OHM + (NWAY + 1) * half
                return blob_t[:, o : o + NWAY + 1]

            # ---- PE warmup (no DMA deps): un-throttle HAM early ------
            nc.vector.memset(scratch_t[:], 0.0)
            warm_ps = [
                ppool.tile([128, 512], f32, tag="ps", name=f"warmps{i}")
                for i in range(2)
            ]
            for i in range(N_WARM_PRE):
                nc.tensor.matmul(
                    warm_ps[i % 2][:], scratch_t[:, 0:128], scratch_t[:],
                    start=True, stop=True,
                )

            # b1 (bf16 cols in blob) -> fp32 for activation bias
            nc.vector.tensor_copy(out=b1f_t[:], in_=blob_t[:, OFF_B1F : OFF_B1F + 2])

            # ---- qpT production interleaved with round-0 H chunks ----
            # qc0's qpT is emitted BEFORE spb (its DMA lands first; spb
            # waiting on the later blob piece must not head-of-line
            # block it on the PE queue).  DVE queue: cast(qc0) -> r0
            # qc0 chunks -> r0 qc1-3.  All other qpT copies ride ACT.
            qpT_t = [spool.tile([128, Q], b16, name=f"qpT{i}") for i in range(2)]
            spb_t = [cpool.tile([128, SP], f32, name=f"spb{i}") for i in range(2)]
            r0_dve = [(j, db) for j in range(4) for db in range(2)
                      if (0, j, db) not in ACT_SET]
            h0_tiles = {}
            for (j, db) in r0_dve:
                h0_tiles[(j, db)] = hpool.tile(
                    [128, Q], b16, tag="Hd", bufs=26, name=f"h{j}_{db}"
                )

            def emit_r0_chunk(lo, hi):
                for (j, db) in r0_dve:
                    nc.vector.tensor_scalar(
                        out=h0_tiles[(j, db)][:, lo:hi],
                        in0=qpT_t[db][:, lo:hi],
                        scalar1=spb_t[db][:, j : j + 1],
                        scalar2=0.0, op0=ADD, op1=MAX,
                    )

            def emit_qpT(qc):
                for db in range(2):
                    qps = ppool.tile([128, QC], f32, tag="ps", name=f"qps{db}{qc}")
                    nc.tensor.matmul(
                        qps[:], w1a(0, db), qT_t[0][:, QC * qc : QC * (qc + 1)],
                        start=True, stop=False,
                    )
                    nc.tensor.matmul(
                        qps[:], w1a(1, db), qT_t[1][:, QC * qc : QC * (qc + 1)],
                        start=False, stop=True,
                    )
                    dst = qpT_t[db][:, QC * qc : QC * (qc + 1)]
                    if qc == 0 and db == 0:
                        nc.vector.tensor_copy(out=dst, in_=qps[:])
                    else:
                        nc.scalar.copy(out=dst, in_=qps[:])

            emit_qpT(0)

            # ---- spbT = W1b^T @ supportT (+b1 on the copy)  [2][128,SP]
            for db in range(2):
                sps = ppool.tile([128, QC], f32, tag="ps", name=f"sps{db}")
                nc.tensor.matmul(sps[:, :SP], w1b(0, db), sT(0), start=True, stop=False)
                nc.tensor.matmul(sps[:, :SP], w1b(1, db), sT(1), start=False, stop=True)
                nc.scalar.activation(
                    spb_t[db][:], sps[:, :SP], IDENT, bias=b1f_t[:, db : db + 1]
                )

            emit_r0_chunk(0, QC)
            for qc in range(1, NQC):
                emit_qpT(qc)
            emit_r0_chunk(QC, 2 * QC)
            emit_r0_chunk(2 * QC, Q)

            # ---- main loop -------------------------------------------
            e_t = spool.tile([128, Q], b16, name="et")
            out_sb = spool.tile([NWAY + 1, Q], b16, name="outsb")
            scores_ps = [
                ppool.tile([128, QC], f32, tag="ps", name=f"sc{qc}")
                for qc in range(NQC)
            ]

            for r in range(NR - 1):
                h_tiles = {}
                for j in range(4):
                    sl = 4 * r + j
                    for db in range(2):
                        if r == 0:
                            h = h0_tiles[(j, db)] if (j, db) in r0_dve else None
                        else:
                            h = None
                        if h is None:
                            if (r, j, db) in ACT_SET:
                                h = hpool.tile(
                                    [128, Q], b16, tag="Ha", bufs=8, name=f"h{sl}_{db}"
                                )
                                nc.scalar.activation(
                                    h[:], qpT_t[db][:], RELU,
                                    bias=spb_t[db][:, sl : sl + 1],
                                )
                            else:
                                h = hpool.tile(
                                    [128, Q], b16, tag="Hd", bufs=26, name=f"h{sl}_{db}"
                                )
                                nc.vector.tensor_scalar(
                                    out=h[:], in0=qpT_t[db][:],
                                    scalar1=spb_t[db][:, sl : sl + 1],
                                    scalar2=0.0, op0=ADD, op1=MAX,
                                )
                        h_tiles[(j, db)] = h
                for db in range(2):
                    for qc in range(NQC):
                        for j in range(4):
                            nc.tensor.matmul(
                                scores_ps[qc][32 * j : 32 * j + 32, :],
                                w2col(db, r),
                                h_tiles[(j, db)][:, QC * qc : QC * (qc + 1)],
                                start=(r == 0 and db == 0),
                                stop=False,
                                tile_position=(0, 32 * j),
                                skip_group_check=True,
                            )
                if r == NR - 2:
                    # pull the LAST round's db1 half forward so only 4
                    # H tiles + 4 matmul groups gate the tail.
                    h9 = []
                    for j in range(4):
                        sl = 4 * (NR - 1) + j
                        h = hpool.tile(
                            [128, Q], b16, tag="Hd", bufs=26, name=f"h{sl}_1"
                        )
                        nc.vector.tensor_scalar(
                            out=h[:], in0=qpT_t[1][:],
                            scalar1=spb_t[1][:, sl : sl + 1],
                            scalar2=0.0, op0=ADD, op1=MAX,
                        )
                        h9.append(h)
                    for qc in range(NQC):
                        for j in range(4):
                            nc.tensor.matmul(
                                scores_ps[qc][32 * j : 32 * j + 32, :],
                                w2col(1, NR - 1),
                                h9[j][:, QC * qc : QC * (qc + 1)],
                                start=False,
                                stop=False,
                                tile_position=(0, 32 * j),
                                skip_group_check=True,
                            )

            # ---- last round (db0 only) + per-qc tail -----------------
            # H in three chunks [qc01 | qc2 | qc3] so qc0-qc2's scores
            # close (and their exps start) before the last H columns.
            r = NR - 1
            hl_tiles = {}
            for j in range(3):
                hl_tiles[j] = hpool.tile(
                    [128, Q], b16, tag="Hd", bufs=26, name=f"h{4 * r + j}_0"
                )
            # j=3 rides ACT full-width: ACT's relu stream drains ~6us
            # before DVE's, so this tile is ready before its matmuls,
            # and DVE's tail-gating chunk stream shrinks by a quarter.
            hl_tiles[3] = hpool.tile(
                [128, Q], b16, tag="Ha", bufs=8, name=f"h{4 * r + 3}_0"
            )
            nc.scalar.activation(
                hl_tiles[3][:], qpT_t[0][:], RELU,
                bias=spb_t[0][:, 4 * r + 3 : 4 * r + 4],
            )
            for (lo, hi) in ((0, 2 * QC), (2 * QC, 3 * QC), (3 * QC, Q)):
                for j in range(3):
                    sl = 4 * r + j
                    nc.vector.tensor_scalar(
                        out=hl_tiles[j][:, lo:hi], in0=qpT_t[0][:, lo:hi],
                        scalar1=spb_t[0][:, sl : sl + 1],
                        scalar2=0.0, op0=ADD, op1=MAX,
                    )
                if lo == 0:
                    # PE warm-keepers pinned to the first chunks (can't
                    # be hoisted): keep HAM at 8/8 into the tail.
                    for i in range(2):
                        wp = ppool.tile([128, 256], f32, tag="ps", name=f"tw{i}")
                        nc.tensor.matmul(
                            wp[:],
                            hl_tiles[2 * i][:, 0:128],
                            hl_tiles[2 * i][:, 0:256],
                            start=True, stop=True,
                        )
            # all four score groups first (PE queue must not
            # head-of-line block later groups behind exp-gated fps
            # matmuls), then the fps/copy/DMA chains per qc.
            for qc in range(NQC):
                for j in range(4):
                    nc.tensor.matmul(
                        scores_ps[qc][32 * j : 32 * j + 32, :],
                        w2col(0, r),
                        hl_tiles[j][:, QC * qc : QC * (qc + 1)],
                        start=False,
                        stop=(j == 3),
                        tile_position=(0, 32 * j),
                        skip_group_check=True,
                    )
                nc.scalar.activation(
                    e_t[:, QC * qc : QC * (qc + 1)], scores_ps[qc][:], EXP,
                )
            for qc in range(NQC):
                fps = ppool.tile([NWAY + 1, QC], f32, tag="ps", name=f"fps{qc}")
                nc.tensor.matmul(
                    fps[:], ohm(0), e_t[:, QC * qc : QC * (qc + 1)],
                    start=True, stop=True,
                )
                dst = out_sb[:, QC * qc : QC * (qc + 1)]
                if qc < 3:
                    nc.vector.tensor_copy(out=dst, in_=fps[:])
                    nc.sync.dma_start(
                        out=out_d[:, QC * qc : QC * (qc + 1)], in_=dst
                    )
                else:
                    # qc3's copy runs on ACT; issuing its DMA on the
                    # same (scalar) ring starts it right after the copy
                    # instead of queueing behind the other three issues
                    # on sync.
                    nc.scalar.copy(out=dst, in_=fps[:])
                    nc.scalar.dma_start(
                        out=out_d[:, QC * qc : QC * (qc + 1)], in_=dst
                    )

    nc.finalize()
    return nc


def _host_prep(inputs):
    """Host-side layout prep: transposes, dtype casts, one-hot tables.

    Returns the list of 8 per-core input dicts for the bass kernel.
    """
    q = np.ascontiguousarray(np.asarray(inputs["query_embeddings"], dtype=np.float32))
    s = np.ascontiguousarray(np.asarray(inputs["support_embeddings"], dtype=np.float32))
    lab = np.asarray(inputs["support_labels"]).astype(np.int64)
    W1 = np.asarray(inputs["W1"], dtype=np.float32)
    b1 = np.asarray(inputs["b1"], dtype=np.float32)
    W2 = np.asarray(inputs["W2"], dtype=np.float32)

    qT = np.ascontiguousarray(q.T).astype(bf16)            # [D, Q]
    sT_full = np.ascontiguousarray(s.T).astype(np.float32) # [D, S]

    blob0 = np.zeros((128, NB), dtype=np.float32)
    for dinb in range(2):
        blob0[:, OFF_W1A + 256 * dinb : OFF_W1A + 256 * (dinb + 1)] = W1[
            128 * dinb : 128 * (dinb + 1)
        ]
        blob0[:, OFF_W1B + 256 * dinb : OFF_W1B + 256 * (dinb + 1)] = W1[
            D + 128 * dinb : D + 128 * (dinb + 1)
        ]
    for db in range(2):
        blk = W2[128 * db : 128 * (db + 1)]
        for r in range(NR):
            blob0[:, OFF_W2C + 32 * (db * NR + r) + r] = blk
        blob0[:, OFF_B1F + db] = b1[128 * db : 128 * (db + 1)]

    in_maps = []
    for c in range(N_CORES):
        lo = c * SP
        blob = blob0.copy()
        for dinb in range(2):
            blob[:, OFF_ST + SP * dinb : OFF_ST + SP * (dinb + 1)] = sT_full[
                128 * dinb : 128 * (dinb + 1), lo : lo + SP
            ]
        for sl in range(SP):
            row = 32 * (sl % 4) + sl // 4
            blob[row, OFF_OHM + lab[lo + sl]] = 1.0
            blob[row, OFF_OHM + NWAY] = 1.0
        in_maps.append({"blob": blob.astype(bf16), "qT": qT})
    return in_maps


def _combine(parts):
    """Sum per-core partials (bf16 on wire) and normalize -> [Q, NWAY] f32."""
    total = np.zeros((NWAY + 1, Q), dtype=np.float32)
    for p in parts:
        total += np.asarray(p, dtype=np.float32)
    return np.ascontiguousarray((total[:NWAY] / total[NWAY : NWAY + 1]).T)


def get_nc():
    global _compiled
    if _compiled is None:
        _compiled = _build_nc()
    return _compiled


def kernel(**inputs) -> np.ndarray:
    from concourse.bass_utils import run_bass_kernel_spmd

    nc = get_nc()
    in_maps = _host_prep(inputs)
    res = run_bass_kernel_spmd(nc, in_maps, list(range(N_CORES)))
    return _combine([res.results[c]["part"] for c in range(N_CORES)])
